# revision 3
# baseline (speedup 1.0000x reference)
"""Deformable 3D convolution (DeformConv3d) on 8 TRN2 NeuronCores via Bass/Tile.

Strategy (data-parallel over the 16 (b, z) output planes, 2 per core):
  - Host packs x into a zero-padded "quad image": for every padded pixel
    (dp, hp, wp) a 128-element row [t=(cy,j) major, c minor] holding the
    2x2 bilinear corner patch across all 32 channels.  One dma_gather
    descriptor then fetches all 4 corners x 32 channels for one
    (tap, sample) pair in a single 512B read.
  - Device per 128-sample chunk: compute floor/frac/corner-weight fields
    on DVE/ACT, build the gather index list (wrapped int16 layout) with
    small PE matmuls, dma_gather spread over 4 SWDGE queues (descriptor
    generation runs on a different Q7 core per queue - the bottleneck -
    measured 454 us/core vs 1147 us on one queue), multiply gathered
    corners by corner weights (DVE), PE-transpose-accumulate the 4
    corners into sampled[(k,c), s] in PSUM, then a 7-step accumulated
    PE matmul against the conv weights, bias-add and store.
"""

import numpy as np
import ml_dtypes

import concourse.bass as bass
import concourse.bacc as bacc
import concourse.mybir as mybir
from concourse import tile
from concourse import library_config
from concourse.bass_utils import run_bass_kernel_spmd
from concourse.tile_rust import add_dep_helper

F32 = mybir.dt.float32
BF16 = mybir.dt.bfloat16
I32 = mybir.dt.int32
I16 = mybir.dt.int16
AT = mybir.AluOpType
AF = mybir.ActivationFunctionType

# problem constants
B, CIN, D, H, W = 2, 32, 8, 48, 48
K, COUT = 27, 64
S = H * W                      # 2304 samples per plane
DP, HPAD, WPAD = 10, 52, 52    # padded depth/rows/cols
PLANE_PX = DP * HPAD * WPAD    # 27040 quad rows per batch
ROW = 128                      # quad row payload elems (4 corners x 32 ch)
NCHUNK = S // 128              # 18
NCOL = 2 * K                   # 54 = (plane, tap) columns per chunk
CALL_COLS = [8, 8, 8, 8, 8, 8, 6]   # dma_gather call split (<=1024 idx each)
N_CORES = 8

_CACHE = {}


def build_nc(debug=False, skip=(), reps=0, nq=4):
    nc = bacc.Bacc("TRN2", target_bir_lowering=False, debug=False,
                   num_swdge_queues=max(2, nq))
    xq = nc.dram_tensor("xq", [PLANE_PX, ROW], F32, kind="ExternalInput")
    offs = nc.dram_tensor("offs", [S, 108], F32, kind="ExternalInput")
    msk = nc.dram_tensor("msk", [S, 54], F32, kind="ExternalInput")
    bases = nc.dram_tensor("bases", [S, 108], F32, kind="ExternalInput")
    dpk = nc.dram_tensor("dpk", [S, 54], F32, kind="ExternalInput")
    wt = nc.dram_tensor("wt", [128, 7 * 64], BF16, kind="ExternalInput")
    bia = nc.dram_tensor("bia", [64, 1], F32, kind="ExternalInput")
    idf = nc.dram_tensor("idf", [128, 128], F32, kind="ExternalInput")
    out = nc.dram_tensor("out", [2, 64, S], F32, kind="ExternalOutput")
    if debug:
        dbg_px = nc.dram_tensor("dbg_px", [128, 54], F32, kind="ExternalOutput")
        dbg_wf = nc.dram_tensor("dbg_wf", [128, 216], F32, kind="ExternalOutput")
        dbg_wr = nc.dram_tensor("dbg_wr", [16, 432], I16, kind="ExternalOutput")
        dbg_g = nc.dram_tensor("dbg_g", [128, NCOL * ROW], F32, kind="ExternalOutput")
        dbg_v = nc.dram_tensor("dbg_v", [128, 8 * 864], F32, kind="ExternalOutput")

    with tile.TileContext(nc) as tc:
        with (
            tc.tile_pool(name="const", bufs=1) as pc,
            tc.tile_pool(name="slab", bufs=3) as ps,
            tc.tile_pool(name="fld", bufs=3) as pf,
            tc.tile_pool(name="gg", bufs=2) as pg,
            tc.tile_pool(name="vv", bufs=2) as pv,
            tc.tile_pool(name="rh", bufs=2) as pr,
            tc.tile_pool(name="oo", bufs=3) as po,
            tc.tile_pool(name="psA", bufs=2, space="PSUM") as pA,
            tc.tile_pool(name="psS", bufs=1, space="PSUM") as pS,
        ):
            wt_t = pc.tile([128, 7 * 64], BF16)
            nc.sync.dma_start(wt_t[:], wt[:])
            bia_t = pc.tile([64, 1], F32)
            nc.sync.dma_start(bia_t[:], bia[:])
            idf_t = pc.tile([128, 128], F32)
            nc.sync.dma_start(idf_t[:], idf[:])
            lib_inst = nc.gpsimd.load_library(library_config.mlp)

            import contextlib
            loop_cm = tc.For_i(0, reps, 1) if reps else contextlib.nullcontext()
            with loop_cm:
              for ci in range(NCHUNK):
                  r0 = ci * 128
                  sl_off = ps.tile([128, 108], F32, tag="off")
                  nc.sync.dma_start(sl_off[:], offs[r0:r0 + 128, :])
                  sl_bas = ps.tile([128, 108], F32, tag="bas")
                  nc.sync.dma_start(sl_bas[:], bases[r0:r0 + 128, :])
                  sl_m = ps.tile([128, 54], F32, tag="m")
                  nc.sync.dma_start(sl_m[:], msk[r0:r0 + 128, :])
                  sl_dpk = ps.tile([128, 54], F32, tag="dpk")
                  nc.sync.dma_start(sl_dpk[:], dpk[r0:r0 + 128, :])

                  # ---- fields: h/w positions, floor, fracs, corner weights
                  hw_ = pf.tile([128, 108], F32, tag="hw")
                  nc.vector.tensor_tensor(out=hw_[:], in0=sl_off[:], in1=sl_bas[:], op=AT.add)
                  nc.vector.tensor_scalar(out=hw_[:], in0=hw_[:], scalar1=49.0,
                                          scalar2=0.0, op0=AT.min, op1=AT.max)
                  ti_ = pf.tile([128, 108], I32, tag="ti")
                  nc.vector.tensor_copy(out=ti_[:], in_=hw_[:])
                  tf_ = pf.tile([128, 108], F32, tag="tf")
                  nc.scalar.activation(out=tf_[:], in_=ti_[:], func=AF.Copy)
                  gt_ = pf.tile([128, 108], F32, tag="gt")
                  nc.vector.tensor_tensor(out=gt_[:], in0=tf_[:], in1=hw_[:], op=AT.is_gt)
                  nc.vector.tensor_tensor(out=tf_[:], in0=tf_[:], in1=gt_[:], op=AT.subtract)
                  l_ = pf.tile([128, 108], F32, tag="l")
                  nc.vector.tensor_tensor(out=l_[:], in0=hw_[:], in1=tf_[:], op=AT.subtract)
                  l1_ = pf.tile([128, 108], F32, tag="l1")
                  nc.scalar.activation(out=l1_[:], in_=l_[:], func=AF.Copy, scale=-1.0, bias=1.0)

                  # px = floor_h * 52 + floor_w + dpk  (exact small ints in f32)
                  px_ = pf.tile([128, 54], F32, tag="px")
                  nc.vector.tensor_scalar(out=px_[:], in0=tf_[:, :54], scalar1=52.0,
                                          scalar2=None, op0=AT.mult)
                  nc.vector.tensor_tensor(out=px_[:], in0=px_[:], in1=tf_[:, 54:], op=AT.add)
                  nc.vector.tensor_tensor(out=px_[:], in0=px_[:], in1=sl_dpk[:], op=AT.add)

                  am_ = pf.tile([128, 54], F32, tag="am")
                  nc.vector.tensor_tensor(out=am_[:], in0=l1_[:, :54], in1=sl_m[:], op=AT.mult)
                  bm_ = pf.tile([128, 54], F32, tag="bm")
                  nc.vector.tensor_tensor(out=bm_[:], in0=l_[:, :54], in1=sl_m[:], op=AT.mult)
                  wf_ = pf.tile([128, 216], F32, tag="wf")
                  for t, (ab, lw0) in enumerate([(am_, l1_), (am_, l_), (bm_, l1_), (bm_, l_)]):
                      nc.vector.tensor_tensor(out=wf_[:, t:216:4], in0=ab[:],
                                              in1=lw0[:, 54:], op=AT.mult)

                  # ---- wrap px into the dma_gather int16 index layout
                  # wrapped[r, col*8+q] = px[q*16+r, col], written into the two
                  # SWDGE queue idx bands (partitions 16:32 / 48:64).
                  wrp = pA.tile([16, 432], F32, tag="wrap", space="PSUM")
                  for q in range(8):
                      nc.tensor.matmul(out=wrp[:, q * 54:(q + 1) * 54],
                                       lhsT=idf_t[:, 16 * q:16 * q + 16],
                                       rhs=px_[:], start=True, stop=True)
                  wr0 = pf.tile([16, 432], I16, tag="wr0")
                  nc.scalar.activation(
                      out=wr0[:].rearrange("p (col q) -> p q col", q=8),
                      in_=wrp[:].rearrange("p (q col) -> p q col", col=54),
                      func=AF.Copy)
                  wrd = pf.tile([128, 432], I16, tag="wrd")
                  nc.vector.memset(wrd[:], 0)
                  # CoreSim's dma_gather reads idxs from partitions 0:16; real
                  # HW's queue-0 generator core reads partitions 16:32.
                  nc.sync.dma_start(wrd[0:16, :], wr0[:])
                  nc.sync.dma_start(wrd[16:32, :], wr0[:])
                  for band0 in (48, 80, 112)[:nq - 1]:
                      nc.sync.dma_start(wrd[band0:band0 + 16, :], wr0[:])

                  # ---- gather: one 512B quad row per (plane, tap, sample)
                  G = pg.tile([128, NCOL * ROW], F32, tag="G")
                  gvw = G[:].rearrange("p (n d) -> p n d", d=ROW)
                  col0 = 0
                  if "gather" in skip:
                      nc.vector.memset(G[:, :1], 0)
                  for calli, ncols in enumerate(CALL_COLS if "gather" not in skip else []):
                      nidx = ncols * 128
                      q = calli % nq
                      gi = nc.gpsimd.dma_gather(
                          gvw[:, col0:col0 + ncols],
                          xq[:],
                          wrd[:, col0 * 8: col0 * 8 + nidx // 16],
                          nidx, nidx, ROW, queue_num=q)
                      add_dep_helper(gi.ins, lib_inst.ins, sync=False,
                                     reason="mlp library before dma_gather")
                      col0 += ncols

                  # ---- corner-weight multiply: V[t,pl][kc, s] (bf16)
                  gq = G[:].rearrange("p (n t c) -> p n t c", t=4, c=32)
                  Vt = pv.tile([128, 8 * 864], F32, tag="V")
                  if "omul" in skip:
                      nc.vector.memset(Vt[:, :1], 0)
                  for pl in (range(2) if "omul" not in skip else []):
                      for t in range(4):
                          vsl = Vt[:, (pl * 4 + t) * 864:(pl * 4 + t + 1) * 864]
                          nc.vector.tensor_tensor(
                              out=vsl.rearrange("p (k c) -> p k c", c=32),
                              in0=gq[:, pl * K:(pl + 1) * K, t, :],
                              in1=wf_[:, pl * 108 + t: pl * 108 + 108: 4].to_broadcast([128, 27, 32]),
                              op=AT.mult)

                  if debug and ci == 0:
                      nc.sync.dma_start(dbg_px[:], px_[:])
                      nc.sync.dma_start(dbg_wf[:], wf_[:])
                      nc.sync.dma_start(dbg_wr[:], wr0[:])
                      nc.sync.dma_start(dbg_g[:], G[:])
                      nc.sync.dma_start(dbg_v[:], Vt[:])

                  # ---- transpose-accumulate corners + conv matmul
                  for pl in (range(2) if "pe" not in skip else []):
                      cp = pA.tile([64, 128], F32, tag="conv", space="PSUM")
                      for g in range(7):
                          gsz = 128 if g < 6 else 96
                          sm = pS.tile([128, 128], F32, tag=f"sm{g % 3}", space="PSUM")
                          for t in range(4):
                              vsl = Vt[:, (pl * 4 + t) * 864 + g * 128:
                                       (pl * 4 + t) * 864 + g * 128 + gsz]
                              nc.tensor.matmul(out=sm[:gsz, :], lhsT=vsl, rhs=idf_t[:],
                                               is_transpose=True,
                                               start=(t == 0), stop=(t == 3))
                          rh = pr.tile([128, 128], BF16, tag=f"rh{g % 3}")
                          nc.scalar.activation(out=rh[:gsz, :], in_=sm[:gsz, :], func=AF.Copy)
                          nc.tensor.matmul(out=cp[:],
                                           lhsT=wt_t[:gsz, g * 64:(g + 1) * 64],
                                           rhs=rh[:gsz, :], start=(g == 0), stop=(g == 6))
                      ou = po.tile([64, 128], F32, tag="ou")
                      nc.vector.tensor_scalar(out=ou[:], in0=cp[:], scalar1=bia_t[:64, :],
                                              scalar2=None, op0=AT.add)
                      nc.sync.dma_start(out[pl, :, r0:r0 + 128], ou[:])

    nc.compile()
    return nc


def _prep_static():
    """Input-independent constant tensors."""
    yy, xx = np.meshgrid(np.arange(H), np.arange(W), indexing="ij")
    yy = yy.reshape(-1).astype(np.float32)
    xx = xx.reshape(-1).astype(np.float32)
    kd = (np.arange(K) // 9).astype(np.float32)
    kh = ((np.arange(K) // 3) % 3).astype(np.float32)
    kw = (np.arange(K) % 3).astype(np.float32)

    bases = np.zeros((S, 108), np.float32)
    for pl in range(2):
        bases[:, pl * K:(pl + 1) * K] = yy[:, None] + kh[None, :]
        bases[:, 54 + pl * K:54 + (pl + 1) * K] = xx[:, None] + kw[None, :]

    idf = np.eye(128, dtype=np.float32)
    return bases, kd, idf


def _prep_weights(weight, bias):
    # wt rows kc = k*32 + c ; wt[kc, o] = weight[o, c, k]
    wk = weight.reshape(COUT, CIN, K)          # [o, c, k]
    wt = np.zeros((896, COUT), np.float32)
    wt[:864] = wk.transpose(2, 1, 0).reshape(864, COUT)   # [k, c, o] -> rows k*32+c
    # pack [7, 128, 64] -> [128, 7*64] for a single contiguous DMA
    wt = wt.reshape(7, 128, COUT).transpose(1, 0, 2).reshape(128, 7 * COUT)
    wt = np.ascontiguousarray(wt).astype(ml_dtypes.bfloat16)
    bia = bias.reshape(64, 1).astype(np.float32)
    return wt, bia


def _prep_quad(x):
    """x [B, C, D, H, W] -> quad [B, PLANE_PX, 128] float32."""
    xp = np.zeros((B, DP, HPAD + 1, WPAD + 1, CIN), np.float32)
    xp[:, 1:1 + D, 1:1 + H, 1:1 + W, :] = x.transpose(0, 2, 3, 4, 1)
    q = np.empty((B, DP, HPAD, WPAD, 4, CIN), np.float32)
    for t, (cy, j) in enumerate([(0, 0), (0, 1), (1, 0), (1, 1)]):
        q[..., t, :] = xp[:, :, cy:cy + HPAD, j:j + WPAD, :]
    return q.reshape(B, PLANE_PX, ROW)


def prepare(input, offset, mask, weight, bias, **build_kw):
    """Build (or reuse) the compiled nc and the per-core input maps."""
    input = np.ascontiguousarray(input, np.float32)
    offset = np.ascontiguousarray(offset, np.float32)
    mask = np.ascontiguousarray(mask, np.float32)
    weight = np.ascontiguousarray(weight, np.float32)
    bias = np.ascontiguousarray(bias, np.float32)

    key = tuple(sorted(build_kw.items()))
    if ("nc", key) not in _CACHE:
        _CACHE[("nc", key)] = build_nc(**build_kw)
    if "static" not in _CACHE:
        _CACHE["static"] = _prep_static()
    nc = _CACHE[("nc", key)]
    bases, kd, idf = _CACHE["static"]
    wt, bia = _prep_weights(weight, bias)
    quad = _prep_quad(input)

    offr = offset.reshape(B, K, 2, D, S)   # [b, k, comp, z, s]
    mr = mask.reshape(B, K, D, S)

    in_maps = []
    for core in range(N_CORES):
        bidx = core // 4
        z0 = (2 * core) % 8
        offs_c = np.empty((S, 108), np.float32)
        msk_c = np.empty((S, 54), np.float32)
        dpk_c = np.empty((S, 54), np.float32)
        for pl, z in enumerate((z0, z0 + 1)):
            offs_c[:, pl * K:(pl + 1) * K] = offr[bidx, :, 0, z, :].T
            offs_c[:, 54 + pl * K:54 + (pl + 1) * K] = offr[bidx, :, 1, z, :].T
            msk_c[:, pl * K:(pl + 1) * K] = mr[bidx, :, z, :].T
            dpk_c[:, pl * K:(pl + 1) * K] = ((z + kd) * (HPAD * WPAD))[None, :]
        in_maps.append({
            "xq": quad[bidx],
            "offs": offs_c,
            "msk": msk_c,
            "bases": bases,
            "dpk": dpk_c,
            "wt": wt,
            "bia": bia,
            "idf": idf,
        })
    return nc, in_maps


def kernel(input, offset, mask, weight, bias):
    nc, in_maps = prepare(input, offset, mask, weight, bias)

    res = run_bass_kernel_spmd(nc, in_maps, core_ids=list(range(N_CORES)))

    out = np.empty((B, COUT, D, H, W), np.float32)
    for core in range(N_CORES):
        bidx = core // 4
        z0 = (2 * core) % 8
        o = np.asarray(res.results[core]["out"], np.float32)   # [2, 64, S]
        out[bidx, :, z0] = o[0].reshape(COUT, H, W)
        out[bidx, :, z0 + 1] = o[1].reshape(COUT, H, W)
    return out



# revision 15
# speedup vs baseline: 1.8867x; 1.8867x over previous
"""Deformable 3D convolution (DeformConv3d) on 8 TRN2 NeuronCores via Bass/Tile.

Strategy (data-parallel over the 16 (b, z) output planes, 2 per core):
  - Host packs x into a zero-padded bf16 "quad image": for every padded pixel
    (dp, hp, wp) a 128-element row [t=(cy,j) major, c minor] holding the
    2x2 bilinear corner patch across all 32 channels.  One dma_gather
    descriptor (256B) fetches all 4 corners x 32 channels for one
    (tap, sample) pair.
  - Three-stage software pipeline over 18 chunks of 128 samples, skewed so
    the SWDGE descriptor generation (the bottleneck) never waits on compute:
      front(i):  load offsets+mask slab, compute floor/frac/corner weights
                 (DVE/ACT), build the wrapped int16 gather index list with
                 small PE matmuls, copy it into the per-queue idx bands.
      mid(i):    4 balanced dma_gather calls, one per SWDGE queue
                 (cols 14/14/13/13 of 54), into bf16 G.
      back(i):   corner-weight multiply (DVE, f32 out), PE-transpose-
                 accumulate the 4 corners into sampled[(k,c), s] (f32 PSUM;
                 bf16 PSUM accumulation is broken on real HW), 7-step
                 accumulated PE matmul against the conv weights (bf16),
                 bias-add and store.

  Measured on the axon trn2 pool: ~310-320 us/pass per core (vs 620 us for
  the unpipelined f32 baseline).  The pace-setter is the gather stage:
  124416 SWDGE descriptors/core/pass across 4 queues (ucode max), ring
  limited to 1024 descriptors (dynamic_dma_scratch_size must stay 16384 -
  bigger rings misbehave on HW).  Deeper pipeline skew (lagm/lagb > (1,2))
  measures consistently worse on HW despite TimelineSim predicting better.
"""

import numpy as np
import ml_dtypes

import concourse.bass as bass
import concourse.bacc as bacc
import concourse.mybir as mybir
from concourse import tile
from concourse import library_config
from concourse.bass_utils import run_bass_kernel_spmd
from concourse.tile_rust import add_dep_helper

F32 = mybir.dt.float32
BF16 = mybir.dt.bfloat16
I32 = mybir.dt.int32
I16 = mybir.dt.int16
AT = mybir.AluOpType
AF = mybir.ActivationFunctionType

# problem constants
B, CIN, D, H, W = 2, 32, 8, 48, 48
K, COUT = 27, 64
S = H * W                      # 2304 samples per plane
DP, HPAD, WPAD = 10, 52, 52    # padded depth/rows/cols
PLANE_PX = DP * HPAD * WPAD    # 27040 quad rows per batch
ROW = 128                      # quad row payload elems (4 corners x 32 ch)
NCHUNK = S // 128              # 18
NCOL = 2 * K                   # 54 = (plane, tap) columns per chunk
CALL_COLS = (7, 7, 7, 7, 7, 7, 6, 6)   # 2 calls/queue/chunk, 14/14/13/13 cols
N_CORES = 8

_CACHE = {}


def build_nc(debug=False, skip=(), reps=0, nq=4, call_cols=CALL_COLS,
             omul_ops=8, floor_mod=False, use_stt=True, scratch=16384,
             tr_f32=True, csum=False, mm_tr=False, lagm=1, lagb=2):
    nc = bacc.Bacc("TRN2", target_bir_lowering=False, debug=False,
                   num_swdge_queues=nq, dynamic_dma_scratch_size=scratch)
    xq = nc.dram_tensor("xq", [PLANE_PX, ROW], BF16, kind="ExternalInput")
    offmsk = nc.dram_tensor("offmsk", [S, 162], F32, kind="ExternalInput")
    dpkt = nc.dram_tensor("dpkt", [128, 54], F32, kind="ExternalInput")
    wt = nc.dram_tensor("wt", [128, 7 * 64], BF16, kind="ExternalInput")
    bia = nc.dram_tensor("bia", [64, 1], F32, kind="ExternalInput")
    idf = nc.dram_tensor("idf", [128, 128], F32, kind="ExternalInput")
    idb = nc.dram_tensor("idb", [128, 128], BF16, kind="ExternalInput")
    out = nc.dram_tensor("out", [2, 64, S], F32, kind="ExternalOutput")
    if debug:
        dbg_px = nc.dram_tensor("dbg_px", [128, 54], F32, kind="ExternalOutput")
        dbg_wf = nc.dram_tensor("dbg_wf", [128, 216], F32, kind="ExternalOutput")
        dbg_wr = nc.dram_tensor("dbg_wr", [16, 432], I16, kind="ExternalOutput")
        dbg_g = nc.dram_tensor("dbg_g", [128, NCOL * ROW], F32, kind="ExternalOutput")
        dbg_v = nc.dram_tensor("dbg_v", [128, 8 * 864], F32, kind="ExternalOutput")

    with tile.TileContext(nc) as tc:
        n_wrd = lagm + 2
        with (
            tc.tile_pool(name="const", bufs=1) as pc,
            tc.tile_pool(name="slab", bufs=3) as ps,
            tc.tile_pool(name="fld", bufs=2) as pf,
            tc.tile_pool(name="wfp", bufs=lagb + 2) as pw,
            tc.tile_pool(name="gg", bufs=lagb - lagm + 2) as pg,
            tc.tile_pool(name="vv", bufs=2) as pv,
            tc.tile_pool(name="rh", bufs=2) as pr,
            tc.tile_pool(name="oo", bufs=3) as po,
            tc.tile_pool(name="psA", bufs=2, space="PSUM") as pA,
            tc.tile_pool(name="psS", bufs=1, space="PSUM") as pS,
        ):
            wt_t = pc.tile([128, 7 * 64], BF16)
            nc.sync.dma_start(wt_t[:], wt[:])
            bia_t = pc.tile([64, 1], F32)
            nc.sync.dma_start(bia_t[:], bia[:])
            idf_t = pc.tile([128, 128], F32)
            nc.sync.dma_start(idf_t[:], idf[:])
            idb_t = pc.tile([128, 128], BF16)
            nc.sync.dma_start(idb_t[:], idb[:])
            dpk_t = pc.tile([128, 54], F32)
            nc.sync.dma_start(dpk_t[:], dpkt[:])
            lib_inst = nc.gpsimd.load_library(library_config.mlp)

            # wrapped idx tiles: one memset each, reused round-robin
            wrd_tiles = [pc.tile([128, 432], I16, tag=f"wrd{j}", name=f"wrd{j}")
                         for j in range(n_wrd)]
            for t in wrd_tiles:
                nc.vector.memset(t[:], 0)

            def front(ci):
                r0 = ci * 128
                slab = ps.tile([128, 162], F32, tag="slab")
                nc.sync.dma_start(slab[:], offmsk[r0:r0 + 128, :])

                hw_ = pf.tile([128, 108], F32, tag="hw")
                nc.vector.tensor_scalar(out=hw_[:], in0=slab[:, :108],
                                        scalar1=49.0, scalar2=0.0,
                                        op0=AT.min, op1=AT.max)
                l_ = pf.tile([128, 108], F32, tag="l")
                tf_ = pf.tile([128, 108], F32, tag="tf")
                if floor_mod:
                    nc.vector.tensor_scalar(out=l_[:], in0=hw_[:], scalar1=1.0,
                                            scalar2=None, op0=AT.mod)
                    nc.vector.tensor_tensor(out=tf_[:], in0=hw_[:], in1=l_[:],
                                            op=AT.subtract)
                else:
                    ti_ = pf.tile([128, 108], I32, tag="ti")
                    nc.vector.tensor_copy(out=ti_[:], in_=hw_[:])
                    nc.scalar.activation(out=tf_[:], in_=ti_[:], func=AF.Copy)
                    gt_ = pf.tile([128, 108], F32, tag="gt")
                    nc.vector.tensor_tensor(out=gt_[:], in0=tf_[:], in1=hw_[:],
                                            op=AT.is_gt)
                    nc.vector.tensor_tensor(out=tf_[:], in0=tf_[:], in1=gt_[:],
                                            op=AT.subtract)
                    nc.vector.tensor_tensor(out=l_[:], in0=hw_[:], in1=tf_[:],
                                            op=AT.subtract)
                l1_ = pf.tile([128, 108], F32, tag="l1")
                nc.scalar.activation(out=l1_[:], in_=l_[:], func=AF.Copy,
                                     scale=-1.0, bias=1.0)

                # px = floor_h * 52 + floor_w + dpk  (exact small ints in f32)
                px_ = pf.tile([128, 54], F32, tag="px")
                if use_stt:
                    nc.vector.scalar_tensor_tensor(out=px_[:], in0=tf_[:, :54],
                                                   scalar=52.0, in1=tf_[:, 54:],
                                                   op0=AT.mult, op1=AT.add)
                else:
                    nc.vector.tensor_scalar(out=px_[:], in0=tf_[:, :54],
                                            scalar1=52.0, scalar2=None,
                                            op0=AT.mult)
                    nc.vector.tensor_tensor(out=px_[:], in0=px_[:],
                                            in1=tf_[:, 54:], op=AT.add)
                nc.vector.tensor_tensor(out=px_[:], in0=px_[:], in1=dpk_t[:],
                                        op=AT.add)

                am_ = pf.tile([128, 54], F32, tag="am")
                nc.vector.tensor_tensor(out=am_[:], in0=l1_[:, :54],
                                        in1=slab[:, 108:], op=AT.mult)
                bm_ = pf.tile([128, 54], F32, tag="bm")
                nc.vector.tensor_tensor(out=bm_[:], in0=l_[:, :54],
                                        in1=slab[:, 108:], op=AT.mult)
                wf_ = pw.tile([128, 216], BF16, tag="wf")
                for t, (ab, lw0) in enumerate([(am_, l1_), (am_, l_),
                                               (bm_, l1_), (bm_, l_)]):
                    nc.vector.tensor_tensor(out=wf_[:, t:216:4], in0=ab[:],
                                            in1=lw0[:, 54:], op=AT.mult)

                # wrap px into the dma_gather int16 index layout:
                # wrapped[r, col*8+q] = px[q*16+r, col]
                wrp = pA.tile([16, 432], F32, tag="wrap", space="PSUM")
                for q in range(8):
                    nc.tensor.matmul(out=wrp[:, q * 54:(q + 1) * 54],
                                     lhsT=idf_t[:, 16 * q:16 * q + 16],
                                     rhs=px_[:], start=True, stop=True)
                wr0 = pf.tile([16, 432], I16, tag="wr0")
                nc.scalar.activation(
                    out=wr0[:].rearrange("p (col q) -> p q col", q=8),
                    in_=wrp[:].rearrange("p (q col) -> p q col", col=54),
                    func=AF.Copy)
                wrd = wrd_tiles[ci % n_wrd]
                # CoreSim's dma_gather reads idxs from partitions 0:16; real
                # HW's queue-q generator core reads partitions q*32+16.
                nc.sync.dma_start(wrd[0:16, :], wr0[:])
                for q in range(nq):
                    nc.sync.dma_start(wrd[q * 32 + 16:q * 32 + 32, :], wr0[:])
                return wrd, wf_

            def mid(ci, wrd):
                G = pg.tile([128, NCOL * ROW], BF16, tag="G")
                gvw = G[:].rearrange("p (n d) -> p n d", d=ROW)
                if "gather" in skip:
                    nc.vector.memset(G[:, :1], 0)
                    return G
                col0 = 0
                for calli, ncols in enumerate(call_cols):
                    nidx = ncols * 128
                    gi = nc.gpsimd.dma_gather(
                        gvw[:, col0:col0 + ncols],
                        xq[:],
                        wrd[:, col0 * 8: col0 * 8 + nidx // 16],
                        nidx, nidx, ROW, queue_num=calli % nq)
                    add_dep_helper(gi.ins, lib_inst.ins, sync=False,
                                   reason="mlp library before dma_gather")
                    col0 += ncols
                return G

            def back(ci, G, wf_):
                r0 = ci * 128
                # corner-weight multiply into (pl, t, k, c) layout
                VDT = F32 if (tr_f32 and not mm_tr) else BF16
                Vt = pv.tile([128, 2 * 4 * 864], VDT, tag="V")
                if "omul" in skip:
                    nc.vector.memset(Vt[:, :1], 0)
                elif omul_ops == 2:
                    gq = G[:].rearrange("p (n kc) -> p n kc", kc=ROW)
                    vq = Vt[:].rearrange("p (pl t k c) -> p pl k t c",
                                         pl=2, t=4, c=32)
                    for pl in range(2):
                        nc.vector.tensor_tensor(
                            out=vq[:, pl],
                            in0=gq[:, pl * K:(pl + 1) * K, :].rearrange(
                                "p k (t c) -> p k t c", c=32),
                            in1=wf_[:, pl * 108:(pl + 1) * 108].rearrange(
                                "p (k t) -> p k t", t=4).to_broadcast(
                                [128, 27, 4, 32]),
                            op=AT.mult)
                else:
                    gq = G[:].rearrange("p (n t c) -> p n t c", t=4, c=32)
                    for pl in range(2):
                        for t in range(4):
                            vsl = Vt[:, (pl * 4 + t) * 864:(pl * 4 + t + 1) * 864]
                            nc.vector.tensor_tensor(
                                out=vsl.rearrange("p (k c) -> p k c", c=32),
                                in0=gq[:, pl * K:(pl + 1) * K, t, :],
                                in1=wf_[:, pl * 108 + t: pl * 108 + 108: 4]
                                    .to_broadcast([128, 27, 32]),
                                op=AT.mult)

                if debug and ci == 0:
                    nc.sync.dma_start(dbg_g[:], G[:])
                    nc.sync.dma_start(dbg_v[:], Vt[:])

                # corner reduction + transpose + conv matmul
                if csum and "pe" not in skip:
                    # DVE pair-sums then one bf16 transpose per group
                    Vs = pv.tile([128, 2 * 864], VDT, tag="Vs")
                    for pl in range(2):
                        t01 = pf.tile([128, 864], VDT, tag="t01")
                        nc.vector.tensor_tensor(
                            out=t01[:], in0=Vt[:, (pl * 4) * 864:(pl * 4 + 1) * 864],
                            in1=Vt[:, (pl * 4 + 1) * 864:(pl * 4 + 2) * 864],
                            op=AT.add)
                        t23 = pf.tile([128, 864], VDT, tag="t23")
                        nc.vector.tensor_tensor(
                            out=t23[:], in0=Vt[:, (pl * 4 + 2) * 864:(pl * 4 + 3) * 864],
                            in1=Vt[:, (pl * 4 + 3) * 864:(pl * 4 + 4) * 864],
                            op=AT.add)
                        nc.vector.tensor_tensor(
                            out=Vs[:, pl * 864:(pl + 1) * 864], in0=t01[:],
                            in1=t23[:], op=AT.add)
                for pl in (range(2) if "pe" not in skip else []):
                    cp = pA.tile([64, 128], F32, tag="conv", space="PSUM")
                    for g in range(7):
                        gsz = 128 if g < 6 else 96
                        sm = pS.tile([128, 128], F32 if mm_tr else VDT,
                                     tag=f"sm{g % 3}", space="PSUM")
                        if csum:
                            nc.tensor.matmul(
                                out=sm[:gsz, :],
                                lhsT=Vs[:, pl * 864 + g * 128:
                                        pl * 864 + g * 128 + gsz],
                                rhs=(idf_t if tr_f32 else idb_t)[:],
                                is_transpose=True, start=True, stop=True)
                        else:
                            for t in range(4):
                                vsl = Vt[:, (pl * 4 + t) * 864 + g * 128:
                                         (pl * 4 + t) * 864 + g * 128 + gsz]
                                if mm_tr:
                                    # regular matmul vs identity: bf16 inputs,
                                    # f32 PSUM accumulate over the 4 corners
                                    nc.tensor.matmul(out=sm[:gsz, :], lhsT=vsl,
                                                     rhs=idb_t[:],
                                                     start=(t == 0), stop=(t == 3))
                                else:
                                    nc.tensor.matmul(out=sm[:gsz, :], lhsT=vsl,
                                                     rhs=(idf_t if tr_f32 else idb_t)[:],
                                                     is_transpose=True,
                                                     start=(t == 0), stop=(t == 3))
                        rh = pr.tile([128, 128], BF16, tag=f"rh{g % 3}")
                        nc.scalar.activation(out=rh[:gsz, :], in_=sm[:gsz, :],
                                             func=AF.Copy)
                        nc.tensor.matmul(out=cp[:],
                                         lhsT=wt_t[:gsz, g * 64:(g + 1) * 64],
                                         rhs=rh[:gsz, :],
                                         start=(g == 0), stop=(g == 6))
                    ou = po.tile([64, 128], F32, tag="ou")
                    nc.vector.tensor_scalar(out=ou[:], in0=cp[:],
                                            scalar1=bia_t[:64, :],
                                            scalar2=None, op0=AT.add)
                    nc.sync.dma_start(out[pl, :, r0:r0 + 128], ou[:])

            import contextlib
            loop_cm = tc.For_i(0, reps, 1) if reps else contextlib.nullcontext()
            with loop_cm:
                live = {}          # ci -> (wrd, wf_, G)
                for i in range(NCHUNK + lagb):
                    if i < NCHUNK:
                        wrd, wf_ = front(i)
                        live[i] = [wrd, wf_, None]
                    if lagm <= i < NCHUNK + lagm:
                        live[i - lagm][2] = mid(i - lagm, live[i - lagm][0])
                    if i >= lagb:
                        _, wf_b, Gb = live.pop(i - lagb)
                        back(i - lagb, Gb, wf_b)

    nc.compile()
    return nc


def _prep_static():
    """Input-independent constant tensors."""
    yy, xx = np.meshgrid(np.arange(H), np.arange(W), indexing="ij")
    yy = yy.reshape(-1).astype(np.float32)
    xx = xx.reshape(-1).astype(np.float32)
    kd = (np.arange(K) // 9).astype(np.float32)
    kh = ((np.arange(K) // 3) % 3).astype(np.float32)
    kw = (np.arange(K) % 3).astype(np.float32)

    bases = np.zeros((S, 108), np.float32)
    for pl in range(2):
        bases[:, pl * K:(pl + 1) * K] = yy[:, None] + kh[None, :]
        bases[:, 54 + pl * K:54 + (pl + 1) * K] = xx[:, None] + kw[None, :]

    idf = np.eye(128, dtype=np.float32)
    idb = np.eye(128, dtype=np.float32).astype(ml_dtypes.bfloat16)
    return bases, kd, idf, idb


def _prep_weights(weight, bias):
    # wt rows kc = k*32 + c ; wt[kc, o] = weight[o, c, k]
    wk = weight.reshape(COUT, CIN, K)          # [o, c, k]
    wt = np.zeros((896, COUT), np.float32)
    wt[:864] = wk.transpose(2, 1, 0).reshape(864, COUT)   # [k, c, o] -> rows k*32+c
    # pack [7, 128, 64] -> [128, 7*64] for a single contiguous DMA
    wt = wt.reshape(7, 128, COUT).transpose(1, 0, 2).reshape(128, 7 * COUT)
    wt = np.ascontiguousarray(wt).astype(ml_dtypes.bfloat16)
    bia = bias.reshape(64, 1).astype(np.float32)
    return wt, bia


def _prep_quad(x):
    """x [B, C, D, H, W] -> quad [B, PLANE_PX, 128] bf16."""
    xp = np.zeros((B, DP, HPAD + 1, WPAD + 1, CIN), np.float32)
    xp[:, 1:1 + D, 1:1 + H, 1:1 + W, :] = x.transpose(0, 2, 3, 4, 1)
    q = np.empty((B, DP, HPAD, WPAD, 4, CIN), np.float32)
    for t, (cy, j) in enumerate([(0, 0), (0, 1), (1, 0), (1, 1)]):
        q[..., t, :] = xp[:, :, cy:cy + HPAD, j:j + WPAD, :]
    return q.reshape(B, PLANE_PX, ROW).astype(ml_dtypes.bfloat16)


def prepare(input, offset, mask, weight, bias, **build_kw):
    """Build (or reuse) the compiled nc and the per-core input maps."""
    input = np.ascontiguousarray(input, np.float32)
    offset = np.ascontiguousarray(offset, np.float32)
    mask = np.ascontiguousarray(mask, np.float32)
    weight = np.ascontiguousarray(weight, np.float32)
    bias = np.ascontiguousarray(bias, np.float32)

    key = tuple(sorted(build_kw.items()))
    if ("nc", key) not in _CACHE:
        _CACHE[("nc", key)] = build_nc(**build_kw)
    if "static" not in _CACHE:
        _CACHE["static"] = _prep_static()
    nc = _CACHE[("nc", key)]
    bases, kd, idf, idb = _CACHE["static"]
    wt, bia = _prep_weights(weight, bias)
    quad = _prep_quad(input)

    offr = offset.reshape(B, K, 2, D, S)   # [b, k, comp, z, s]
    mr = mask.reshape(B, K, D, S)

    in_maps = []
    for core in range(N_CORES):
        bidx = core // 4
        z0 = (2 * core) % 8
        offmsk_c = np.empty((S, 162), np.float32)
        dpk_c = np.empty((1, 54), np.float32)
        for pl, z in enumerate((z0, z0 + 1)):
            offmsk_c[:, pl * K:(pl + 1) * K] = offr[bidx, :, 0, z, :].T
            offmsk_c[:, 54 + pl * K:54 + (pl + 1) * K] = offr[bidx, :, 1, z, :].T
            offmsk_c[:, 108 + pl * K:108 + (pl + 1) * K] = mr[bidx, :, z, :].T
            dpk_c[0, pl * K:(pl + 1) * K] = (z + kd) * (HPAD * WPAD)
        offmsk_c[:, :108] += bases
        in_maps.append({
            "xq": quad[bidx],
            "offmsk": offmsk_c,
            "dpkt": np.broadcast_to(dpk_c, (128, 54)).copy(),
            "wt": wt,
            "bia": bia,
            "idf": idf,
            "idb": idb,
        })
    return nc, in_maps


def kernel(input, offset, mask, weight, bias):
    nc, in_maps = prepare(input, offset, mask, weight, bias)

    res = run_bass_kernel_spmd(nc, in_maps, core_ids=list(range(N_CORES)))

    out = np.empty((B, COUT, D, H, W), np.float32)
    for core in range(N_CORES):
        bidx = core // 4
        z0 = (2 * core) % 8
        o = np.asarray(res.results[core]["out"], np.float32)   # [2, 64, S]
        out[bidx, :, z0] = o[0].reshape(COUT, H, W)
        out[bidx, :, z0 + 1] = o[1].reshape(COUT, H, W)
    return out


# revision 17
# speedup vs baseline: 1.9510x; 1.0341x over previous
"""Deformable 3D convolution (DeformConv3d) on 8 TRN2 NeuronCores via Bass/Tile.

Strategy (data-parallel over the 16 (b, z) output planes, 2 per core):
  - Host packs x into a zero-padded bf16 "quad image": for every padded pixel
    (dp, hp, wp) a 128-element row [t=(cy,j) major, c minor] holding the
    2x2 bilinear corner patch across all 32 channels.  One dma_gather
    descriptor (256B) fetches all 4 corners x 32 channels for one
    (tap, sample) pair.
  - Three-stage software pipeline over 18 chunks of 128 samples, skewed so
    the SWDGE descriptor generation (the bottleneck) never waits on compute:
      front(i):  load offsets+mask slab, compute floor/frac/corner weights
                 (DVE/ACT), build the wrapped int16 gather index list with
                 small PE matmuls, copy it into the per-queue idx bands.
      mid(i):    4 balanced dma_gather calls, one per SWDGE queue
                 (cols 14/14/13/13 of 54), into bf16 G.
      back(i):   corner-weight multiply (DVE, f32 out), PE-transpose-
                 accumulate the 4 corners into sampled[(k,c), s] (f32 PSUM;
                 bf16 PSUM accumulation is broken on real HW), 7-step
                 accumulated PE matmul against the conv weights (bf16),
                 bias-add and store.

  Measured on the axon trn2 pool: ~310-320 us/pass per core (vs 620 us for
  the unpipelined f32 baseline).  The pace-setter is the gather stage:
  124416 SWDGE descriptors/core/pass across 4 queues (ucode max), ring
  limited to 1024 descriptors (dynamic_dma_scratch_size must stay 16384 -
  bigger rings misbehave on HW).  Deeper pipeline skew (lagm/lagb > (1,2))
  measures consistently worse on HW despite TimelineSim predicting better.
"""

import numpy as np
import ml_dtypes

import concourse.bass as bass
import concourse.bacc as bacc
import concourse.mybir as mybir
from concourse import tile
from concourse import library_config
from concourse.bass_utils import run_bass_kernel_spmd
from concourse.tile_rust import add_dep_helper

F32 = mybir.dt.float32
BF16 = mybir.dt.bfloat16
I32 = mybir.dt.int32
I16 = mybir.dt.int16
AT = mybir.AluOpType
AF = mybir.ActivationFunctionType

# problem constants
B, CIN, D, H, W = 2, 32, 8, 48, 48
K, COUT = 27, 64
S = H * W                      # 2304 samples per plane
DP, HPAD, WPAD = 10, 52, 52    # padded depth/rows/cols
PLANE_PX = DP * HPAD * WPAD    # 27040 quad rows per batch
ROW = 128                      # quad row payload elems (4 corners x 32 ch)
NCHUNK = S // 128              # 18
NCOL = 2 * K                   # 54 = (plane, tap) columns per chunk
CALL_COLS = (7, 7, 7, 7, 7, 7, 6, 6)   # 2 calls/queue/chunk, 14/14/13/13 cols
N_CORES = 8

_CACHE = {}


def build_nc(debug=False, skip=(), reps=0, nq=4, call_cols=CALL_COLS,
             omul_ops=8, floor_mod=False, use_stt=True, scratch=16384,
             tr_f32=True, csum=False, mm_tr=False, lagm=1, lagb=2,
             sim_bands=False, merge_st=True):
    nc = bacc.Bacc("TRN2", target_bir_lowering=False, debug=False,
                   num_swdge_queues=nq, dynamic_dma_scratch_size=scratch)
    xq = nc.dram_tensor("xq", [PLANE_PX, ROW], BF16, kind="ExternalInput")
    offmsk = nc.dram_tensor("offmsk", [S, 162], F32, kind="ExternalInput")
    dpkt = nc.dram_tensor("dpkt", [128, 54], F32, kind="ExternalInput")
    wt = nc.dram_tensor("wt", [128, 7 * 64], BF16, kind="ExternalInput")
    bia = nc.dram_tensor("bia", [64, 1], F32, kind="ExternalInput")
    idf = nc.dram_tensor("idf", [128, 128], F32, kind="ExternalInput")
    idb = nc.dram_tensor("idb", [128, 128], BF16, kind="ExternalInput")
    out = nc.dram_tensor("out", [2, 64, S], F32, kind="ExternalOutput")
    if debug:
        dbg_px = nc.dram_tensor("dbg_px", [128, 54], F32, kind="ExternalOutput")
        dbg_wf = nc.dram_tensor("dbg_wf", [128, 216], F32, kind="ExternalOutput")
        dbg_wr = nc.dram_tensor("dbg_wr", [16, 432], I16, kind="ExternalOutput")
        dbg_g = nc.dram_tensor("dbg_g", [128, NCOL * ROW], F32, kind="ExternalOutput")
        dbg_v = nc.dram_tensor("dbg_v", [128, 8 * 864], F32, kind="ExternalOutput")

    with tile.TileContext(nc) as tc:
        n_wrd = lagm + 2
        with (
            tc.tile_pool(name="const", bufs=1) as pc,
            tc.tile_pool(name="slab", bufs=3) as ps,
            tc.tile_pool(name="fld", bufs=2) as pf,
            tc.tile_pool(name="wfp", bufs=lagb + 2) as pw,
            tc.tile_pool(name="gg", bufs=lagb - lagm + 2) as pg,
            tc.tile_pool(name="vv", bufs=2) as pv,
            tc.tile_pool(name="rh", bufs=2) as pr,
            tc.tile_pool(name="oo", bufs=3) as po,
            tc.tile_pool(name="psA", bufs=2, space="PSUM") as pA,
            tc.tile_pool(name="psS", bufs=1, space="PSUM") as pS,
        ):
            wt_t = pc.tile([128, 7 * 64], BF16)
            nc.sync.dma_start(wt_t[:], wt[:])
            bia_t = pc.tile([64, 1], F32)
            nc.sync.dma_start(bia_t[:], bia[:])
            idf_t = pc.tile([128, 128], F32)
            nc.sync.dma_start(idf_t[:], idf[:])
            idb_t = pc.tile([128, 128], BF16)
            nc.sync.dma_start(idb_t[:], idb[:])
            dpk_t = pc.tile([128, 54], F32)
            nc.sync.dma_start(dpk_t[:], dpkt[:])
            lib_inst = nc.gpsimd.load_library(library_config.mlp)

            # wrapped idx tiles: one memset each, reused round-robin
            wrd_tiles = [pc.tile([128, 432], I16, tag=f"wrd{j}", name=f"wrd{j}")
                         for j in range(n_wrd)]
            for t in wrd_tiles:
                nc.vector.memset(t[:], 0)

            def front(ci):
                r0 = ci * 128
                slab = ps.tile([128, 162], F32, tag="slab")
                nc.sync.dma_start(slab[:], offmsk[r0:r0 + 128, :])

                hw_ = pf.tile([128, 108], F32, tag="hw")
                nc.vector.tensor_scalar(out=hw_[:], in0=slab[:, :108],
                                        scalar1=49.0, scalar2=0.0,
                                        op0=AT.min, op1=AT.max)
                l_ = pf.tile([128, 108], F32, tag="l")
                tf_ = pf.tile([128, 108], F32, tag="tf")
                if floor_mod:
                    nc.vector.tensor_scalar(out=l_[:], in0=hw_[:], scalar1=1.0,
                                            scalar2=None, op0=AT.mod)
                    nc.vector.tensor_tensor(out=tf_[:], in0=hw_[:], in1=l_[:],
                                            op=AT.subtract)
                else:
                    ti_ = pf.tile([128, 108], I32, tag="ti")
                    nc.vector.tensor_copy(out=ti_[:], in_=hw_[:])
                    nc.scalar.activation(out=tf_[:], in_=ti_[:], func=AF.Copy)
                    gt_ = pf.tile([128, 108], F32, tag="gt")
                    nc.vector.tensor_tensor(out=gt_[:], in0=tf_[:], in1=hw_[:],
                                            op=AT.is_gt)
                    nc.vector.tensor_tensor(out=tf_[:], in0=tf_[:], in1=gt_[:],
                                            op=AT.subtract)
                    nc.vector.tensor_tensor(out=l_[:], in0=hw_[:], in1=tf_[:],
                                            op=AT.subtract)
                l1_ = pf.tile([128, 108], F32, tag="l1")
                nc.scalar.activation(out=l1_[:], in_=l_[:], func=AF.Copy,
                                     scale=-1.0, bias=1.0)

                # px = floor_h * 52 + floor_w + dpk  (exact small ints in f32)
                px_ = pf.tile([128, 54], F32, tag="px")
                if use_stt:
                    nc.vector.scalar_tensor_tensor(out=px_[:], in0=tf_[:, :54],
                                                   scalar=52.0, in1=tf_[:, 54:],
                                                   op0=AT.mult, op1=AT.add)
                else:
                    nc.vector.tensor_scalar(out=px_[:], in0=tf_[:, :54],
                                            scalar1=52.0, scalar2=None,
                                            op0=AT.mult)
                    nc.vector.tensor_tensor(out=px_[:], in0=px_[:],
                                            in1=tf_[:, 54:], op=AT.add)
                nc.vector.tensor_tensor(out=px_[:], in0=px_[:], in1=dpk_t[:],
                                        op=AT.add)

                am_ = pf.tile([128, 54], F32, tag="am")
                nc.vector.tensor_tensor(out=am_[:], in0=l1_[:, :54],
                                        in1=slab[:, 108:], op=AT.mult)
                bm_ = pf.tile([128, 54], F32, tag="bm")
                nc.vector.tensor_tensor(out=bm_[:], in0=l_[:, :54],
                                        in1=slab[:, 108:], op=AT.mult)
                wf_ = pw.tile([128, 216], BF16, tag="wf")
                for t, (ab, lw0) in enumerate([(am_, l1_), (am_, l_),
                                               (bm_, l1_), (bm_, l_)]):
                    nc.vector.tensor_tensor(out=wf_[:, t:216:4], in0=ab[:],
                                            in1=lw0[:, 54:], op=AT.mult)

                # wrap px into the dma_gather int16 index layout:
                # wrapped[r, col*8+q] = px[q*16+r, col]
                wrp = pA.tile([16, 432], F32, tag="wrap", space="PSUM")
                for q in range(8):
                    nc.tensor.matmul(out=wrp[:, q * 54:(q + 1) * 54],
                                     lhsT=idf_t[:, 16 * q:16 * q + 16],
                                     rhs=px_[:], start=True, stop=True)
                wr0 = pf.tile([16, 432], I16, tag="wr0")
                nc.scalar.activation(
                    out=wr0[:].rearrange("p (col q) -> p q col", q=8),
                    in_=wrp[:].rearrange("p (q col) -> p q col", col=54),
                    func=AF.Copy)
                wrd = wrd_tiles[ci % n_wrd]
                # CoreSim's dma_gather reads idxs from partitions 0:16; real
                # HW's queue-q generator core reads partitions q*32+16.
                if sim_bands:
                    nc.sync.dma_start(wrd[0:16, :], wr0[:])
                for q in range(nq):
                    nc.sync.dma_start(wrd[q * 32 + 16:q * 32 + 32, :], wr0[:])
                return wrd, wf_

            def mid(ci, wrd):
                G = pg.tile([128, NCOL * ROW], BF16, tag="G")
                gvw = G[:].rearrange("p (n d) -> p n d", d=ROW)
                if "gather" in skip:
                    nc.vector.memset(G[:, :1], 0)
                    return G
                col0 = 0
                for calli, ncols in enumerate(call_cols):
                    nidx = ncols * 128
                    gi = nc.gpsimd.dma_gather(
                        gvw[:, col0:col0 + ncols],
                        xq[:],
                        wrd[:, col0 * 8: col0 * 8 + nidx // 16],
                        nidx, nidx, ROW, queue_num=calli % nq)
                    add_dep_helper(gi.ins, lib_inst.ins, sync=False,
                                   reason="mlp library before dma_gather")
                    col0 += ncols
                return G

            def back(ci, G, wf_):
                r0 = ci * 128
                # corner-weight multiply into (pl, t, k, c) layout
                VDT = F32 if (tr_f32 and not mm_tr) else BF16
                Vt = pv.tile([128, 2 * 4 * 864], VDT, tag="V")
                if "omul" in skip:
                    nc.vector.memset(Vt[:, :1], 0)
                elif omul_ops == 2:
                    gq = G[:].rearrange("p (n kc) -> p n kc", kc=ROW)
                    vq = Vt[:].rearrange("p (pl t k c) -> p pl k t c",
                                         pl=2, t=4, c=32)
                    for pl in range(2):
                        nc.vector.tensor_tensor(
                            out=vq[:, pl],
                            in0=gq[:, pl * K:(pl + 1) * K, :].rearrange(
                                "p k (t c) -> p k t c", c=32),
                            in1=wf_[:, pl * 108:(pl + 1) * 108].rearrange(
                                "p (k t) -> p k t", t=4).to_broadcast(
                                [128, 27, 4, 32]),
                            op=AT.mult)
                else:
                    gq = G[:].rearrange("p (n t c) -> p n t c", t=4, c=32)
                    for pl in range(2):
                        for t in range(4):
                            vsl = Vt[:, (pl * 4 + t) * 864:(pl * 4 + t + 1) * 864]
                            nc.vector.tensor_tensor(
                                out=vsl.rearrange("p (k c) -> p k c", c=32),
                                in0=gq[:, pl * K:(pl + 1) * K, t, :],
                                in1=wf_[:, pl * 108 + t: pl * 108 + 108: 4]
                                    .to_broadcast([128, 27, 32]),
                                op=AT.mult)

                if debug and ci == 0:
                    nc.sync.dma_start(dbg_g[:], G[:])
                    nc.sync.dma_start(dbg_v[:], Vt[:])

                # corner reduction + transpose + conv matmul
                if csum and "pe" not in skip:
                    # DVE pair-sums then one bf16 transpose per group
                    Vs = pv.tile([128, 2 * 864], VDT, tag="Vs")
                    for pl in range(2):
                        t01 = pf.tile([128, 864], VDT, tag="t01")
                        nc.vector.tensor_tensor(
                            out=t01[:], in0=Vt[:, (pl * 4) * 864:(pl * 4 + 1) * 864],
                            in1=Vt[:, (pl * 4 + 1) * 864:(pl * 4 + 2) * 864],
                            op=AT.add)
                        t23 = pf.tile([128, 864], VDT, tag="t23")
                        nc.vector.tensor_tensor(
                            out=t23[:], in0=Vt[:, (pl * 4 + 2) * 864:(pl * 4 + 3) * 864],
                            in1=Vt[:, (pl * 4 + 3) * 864:(pl * 4 + 4) * 864],
                            op=AT.add)
                        nc.vector.tensor_tensor(
                            out=Vs[:, pl * 864:(pl + 1) * 864], in0=t01[:],
                            in1=t23[:], op=AT.add)
                ou2 = (po.tile([64, 256], F32, tag="ou2", name="ou2")
                       if merge_st else None)
                for pl in (range(2) if "pe" not in skip else []):
                    cp = pA.tile([64, 128], F32, tag="conv", space="PSUM")
                    for g in range(7):
                        gsz = 128 if g < 6 else 96
                        sm = pS.tile([128, 128], F32 if mm_tr else VDT,
                                     tag=f"sm{g % 3}", space="PSUM")
                        if csum:
                            nc.tensor.matmul(
                                out=sm[:gsz, :],
                                lhsT=Vs[:, pl * 864 + g * 128:
                                        pl * 864 + g * 128 + gsz],
                                rhs=(idf_t if tr_f32 else idb_t)[:],
                                is_transpose=True, start=True, stop=True)
                        else:
                            for t in range(4):
                                vsl = Vt[:, (pl * 4 + t) * 864 + g * 128:
                                         (pl * 4 + t) * 864 + g * 128 + gsz]
                                if mm_tr:
                                    # regular matmul vs identity: bf16 inputs,
                                    # f32 PSUM accumulate over the 4 corners
                                    nc.tensor.matmul(out=sm[:gsz, :], lhsT=vsl,
                                                     rhs=idb_t[:],
                                                     start=(t == 0), stop=(t == 3))
                                else:
                                    nc.tensor.matmul(out=sm[:gsz, :], lhsT=vsl,
                                                     rhs=(idf_t if tr_f32 else idb_t)[:],
                                                     is_transpose=True,
                                                     start=(t == 0), stop=(t == 3))
                        rh = pr.tile([128, 128], BF16, tag=f"rh{g % 3}")
                        nc.scalar.activation(out=rh[:gsz, :], in_=sm[:gsz, :],
                                             func=AF.Copy)
                        nc.tensor.matmul(out=cp[:],
                                         lhsT=wt_t[:gsz, g * 64:(g + 1) * 64],
                                         rhs=rh[:gsz, :],
                                         start=(g == 0), stop=(g == 6))
                    if merge_st:
                        nc.vector.tensor_scalar(out=ou2[:, pl * 128:(pl + 1) * 128],
                                                in0=cp[:], scalar1=bia_t[:64, :],
                                                scalar2=None, op0=AT.add)
                    else:
                        ou = po.tile([64, 128], F32, tag="ou")
                        nc.vector.tensor_scalar(out=ou[:], in0=cp[:],
                                                scalar1=bia_t[:64, :],
                                                scalar2=None, op0=AT.add)
                        nc.sync.dma_start(out[pl, :, r0:r0 + 128], ou[:])
                if merge_st and "pe" not in skip:
                    nc.sync.dma_start(
                        out[:, :, r0:r0 + 128].rearrange("pl c s -> c pl s"),
                        ou2[:].rearrange("c (pl s) -> c pl s", s=128))

            import contextlib
            loop_cm = tc.For_i(0, reps, 1) if reps else contextlib.nullcontext()
            with loop_cm:
                live = {}          # ci -> (wrd, wf_, G)
                for i in range(NCHUNK + lagb):
                    if i < NCHUNK:
                        wrd, wf_ = front(i)
                        live[i] = [wrd, wf_, None]
                    if lagm <= i < NCHUNK + lagm:
                        live[i - lagm][2] = mid(i - lagm, live[i - lagm][0])
                    if i >= lagb:
                        _, wf_b, Gb = live.pop(i - lagb)
                        back(i - lagb, Gb, wf_b)

    nc.compile()
    return nc


def _prep_static():
    """Input-independent constant tensors."""
    yy, xx = np.meshgrid(np.arange(H), np.arange(W), indexing="ij")
    yy = yy.reshape(-1).astype(np.float32)
    xx = xx.reshape(-1).astype(np.float32)
    kd = (np.arange(K) // 9).astype(np.float32)
    kh = ((np.arange(K) // 3) % 3).astype(np.float32)
    kw = (np.arange(K) % 3).astype(np.float32)

    bases = np.zeros((S, 108), np.float32)
    for pl in range(2):
        bases[:, pl * K:(pl + 1) * K] = yy[:, None] + kh[None, :]
        bases[:, 54 + pl * K:54 + (pl + 1) * K] = xx[:, None] + kw[None, :]

    idf = np.eye(128, dtype=np.float32)
    idb = np.eye(128, dtype=np.float32).astype(ml_dtypes.bfloat16)
    return bases, kd, idf, idb


def _prep_weights(weight, bias):
    # wt rows kc = k*32 + c ; wt[kc, o] = weight[o, c, k]
    wk = weight.reshape(COUT, CIN, K)          # [o, c, k]
    wt = np.zeros((896, COUT), np.float32)
    wt[:864] = wk.transpose(2, 1, 0).reshape(864, COUT)   # [k, c, o] -> rows k*32+c
    # pack [7, 128, 64] -> [128, 7*64] for a single contiguous DMA
    wt = wt.reshape(7, 128, COUT).transpose(1, 0, 2).reshape(128, 7 * COUT)
    wt = np.ascontiguousarray(wt).astype(ml_dtypes.bfloat16)
    bia = bias.reshape(64, 1).astype(np.float32)
    return wt, bia


def _prep_quad(x):
    """x [B, C, D, H, W] -> quad [B, PLANE_PX, 128] bf16."""
    xp = np.zeros((B, DP, HPAD + 1, WPAD + 1, CIN), np.float32)
    xp[:, 1:1 + D, 1:1 + H, 1:1 + W, :] = x.transpose(0, 2, 3, 4, 1)
    q = np.empty((B, DP, HPAD, WPAD, 4, CIN), np.float32)
    for t, (cy, j) in enumerate([(0, 0), (0, 1), (1, 0), (1, 1)]):
        q[..., t, :] = xp[:, :, cy:cy + HPAD, j:j + WPAD, :]
    return q.reshape(B, PLANE_PX, ROW).astype(ml_dtypes.bfloat16)


def prepare(input, offset, mask, weight, bias, **build_kw):
    """Build (or reuse) the compiled nc and the per-core input maps."""
    input = np.ascontiguousarray(input, np.float32)
    offset = np.ascontiguousarray(offset, np.float32)
    mask = np.ascontiguousarray(mask, np.float32)
    weight = np.ascontiguousarray(weight, np.float32)
    bias = np.ascontiguousarray(bias, np.float32)

    key = tuple(sorted(build_kw.items()))
    if ("nc", key) not in _CACHE:
        _CACHE[("nc", key)] = build_nc(**build_kw)
    if "static" not in _CACHE:
        _CACHE["static"] = _prep_static()
    nc = _CACHE[("nc", key)]
    bases, kd, idf, idb = _CACHE["static"]
    wt, bia = _prep_weights(weight, bias)
    quad = _prep_quad(input)

    offr = offset.reshape(B, K, 2, D, S)   # [b, k, comp, z, s]
    mr = mask.reshape(B, K, D, S)

    in_maps = []
    for core in range(N_CORES):
        bidx = core // 4
        z0 = (2 * core) % 8
        offmsk_c = np.empty((S, 162), np.float32)
        dpk_c = np.empty((1, 54), np.float32)
        for pl, z in enumerate((z0, z0 + 1)):
            offmsk_c[:, pl * K:(pl + 1) * K] = offr[bidx, :, 0, z, :].T
            offmsk_c[:, 54 + pl * K:54 + (pl + 1) * K] = offr[bidx, :, 1, z, :].T
            offmsk_c[:, 108 + pl * K:108 + (pl + 1) * K] = mr[bidx, :, z, :].T
            dpk_c[0, pl * K:(pl + 1) * K] = (z + kd) * (HPAD * WPAD)
        offmsk_c[:, :108] += bases
        in_maps.append({
            "xq": quad[bidx],
            "offmsk": offmsk_c,
            "dpkt": np.broadcast_to(dpk_c, (128, 54)).copy(),
            "wt": wt,
            "bia": bia,
            "idf": idf,
            "idb": idb,
        })
    return nc, in_maps


def kernel(input, offset, mask, weight, bias):
    nc, in_maps = prepare(input, offset, mask, weight, bias)

    res = run_bass_kernel_spmd(nc, in_maps, core_ids=list(range(N_CORES)))

    out = np.empty((B, COUT, D, H, W), np.float32)
    for core in range(N_CORES):
        bidx = core // 4
        z0 = (2 * core) % 8
        o = np.asarray(res.results[core]["out"], np.float32)   # [2, 64, S]
        out[bidx, :, z0] = o[0].reshape(COUT, H, W)
        out[bidx, :, z0 + 1] = o[1].reshape(COUT, H, W)
    return out
